# revision 23
# baseline (speedup 1.0000x reference)
"""Trainium2 Bass kernel for nn_DART_Net (gnn_message_passing).

Math (per molecule b, node n):
  hi = mlp2_i(ai) * mask(ai)                 [B,N,128]
  h{j,k,l} = mlp2_t(a_t) * mask(a_t)        [B,N,M,128] -> sum over M
  atm = hi + sum_j + sum_k + sum_l
  out = (celu-chain(atm) @ W4.T + b4) * mask(ai)
with mlp2(x) = celu(celu(x@W1.T+b1)@W2.T+b2), celu alpha=0.1.

Device strategy (per core, pure data parallel over B):
  - features on partitions, message rows on the free axis
  - celu(z+b) == max(z+b, min(alpha*e^((z+b)/alpha), alpha) - alpha)
      ACT:  t = Exp(z/alpha + (b/alpha + ln(alpha)))   (= alpha*e^((z+b)/alpha))
      DVE:  v = (t min alpha) sub alpha                (tensor_scalar dual-op)
      DVE:  e = (z add b) max v                        (scalar_tensor_tensor)
  - sum over M: trunk layer-1 is linear, so W1c @ sum_m e2 == sum_m W1c @ e2;
    accumulate straight into a phase-resident PSUM bank with a repeated
    (step-0) matmul output access pattern (fan-in).  Fallback: DVE reduce.
  - pad masks: ai mask applied at the output on device; exact-zero message
    rows (probability ~0 with randn inputs) are fixed up via a host-computed
    per-node additive correction "corr" that enters the same trunk matmul.
"""

import sys
import numpy as np
from contextlib import ExitStack

for _p in ("/opt/trn_rl_repo", "/root/.axon_site/_ro/trn_rl_repo"):
    if _p not in sys.path:
        sys.path.append(_p)

ALPHA = 0.1
INV_ALPHA = 1.0 / ALPHA
LN_ALPHA = float(np.log(np.float32(ALPHA)))

B, N, M = 64, 128, 64
NCORES = 8
BL = B // NCORES          # molecules per core
LH, LO = 128, 128
C1, C2, C3 = 64, 32, 16

USE_FANIN = True         # PE fan-in m-reduce; False -> DVE tensor_reduce
TS_ON_GPSIMD = False     # run the tensor_scalar (v = min(t,a)-a) on GPSIMD
SPLIT1 = False           # celu1 = Relu(z+b) + v via two accumulating L2 matmuls
MERGE2 = True            # single [128,1024] L2 psum; one exp2/TS2/STT2 per group
STRIDED_RHS = False       # fan-in matmul reads e2 strided; e2 written contiguous

_PROGRAM_CACHE = {}


# --------------------------------------------------------------------------
# device program
# --------------------------------------------------------------------------

def _build_program(nmol=BL, debug=False, use_fanin=USE_FANIN, reps=1,
                   ts_gpsimd=None, abl=None):
    # abl: None | "nodve" | "noact" | "nodma"  (timing ablations, wrong math)
    if ts_gpsimd is None:
        ts_gpsimd = TS_ON_GPSIMD
    import concourse.bass as bass
    import concourse.tile as tile
    from concourse import bacc, mybir

    f32 = mybir.dt.float32
    Alu = mybir.AluOpType
    Act = mybir.ActivationFunctionType

    nodes = nmol * N                  # nodes per core
    rmsg = nodes * M                  # message rows per tensor per core
    PH = min(512, nodes)              # nodes per trunk phase (1 PSUM bank)
    nphase = nodes // PH
    G = 1024                          # message columns per group
    rows_ph = PH * M                  # message rows per phase per tensor
    ngrp = rows_ph // G
    npg = G // M                      # nodes per group (16)

    nc = bacc.Bacc("TRN2", target_bir_lowering=False, debug=debug)

    x_dram = {t: nc.dram_tensor(f"x{t}", [3, rmsg], f32, kind="ExternalInput")
              for t in "jkl"}
    xi_dram = nc.dram_tensor("xi", [3, nodes], f32, kind="ExternalInput")
    corr_dram = nc.dram_tensor("corr", [LO, nodes], f32, kind="ExternalInput")
    mi_dram = nc.dram_tensor("mi", [1, nodes], f32, kind="ExternalInput")

    w_specs = {}
    for t in "jkli":
        w_specs[f"w1{t}"] = [3, LH]
        w_specs[f"w2{t}"] = [LH, LO]
        w_specs[f"eb1{t}"] = [LH, 1]
        w_specs[f"rb1{t}"] = [LH, 1]
        w_specs[f"eb2{t}"] = [LO, 1]
        w_specs[f"rb2{t}"] = [LO, 1]
    w_specs.update(wc1=[LO, C1], ebc1=[C1, 1], rbc1=[C1, 1],
                   wc2=[C1, C2], ebc2=[C2, 1], rbc2=[C2, 1],
                   wc3=[C2, C3], ebc3=[C3, 1], rbc3=[C3, 1],
                   wc4=[C3, 1], bc4=[1, 1])
    w_dram = {k: nc.dram_tensor(k, v, f32, kind="ExternalInput")
              for k, v in w_specs.items()}
    out_dram = nc.dram_tensor("out", [1, nodes], f32, kind="ExternalOutput")

    with ExitStack() as ctx:
        tc = ctx.enter_context(tile.TileContext(nc))

        wpool = ctx.enter_context(tc.tile_pool(name="w", bufs=1))
        xpool = ctx.enter_context(tc.tile_pool(name="x", bufs=4))
        if MERGE2:
            # one shared psum pool: [128,1024] tiles (2 banks) x 3 bufs
            z_pool = ctx.enter_context(tc.tile_pool(name="z", bufs=3,
                                                    space="PSUM"))
            za_pool = zb_pool = z_pool
        else:
            za_pool = ctx.enter_context(tc.tile_pool(name="za", bufs=2,
                                                     space="PSUM"))
            zb_pool = ctx.enter_context(tc.tile_pool(name="zb", bufs=3,
                                                     space="PSUM"))
        tr_pool = ctx.enter_context(tc.tile_pool(name="tr", bufs=1, space="PSUM"))
        t1_pool = ctx.enter_context(tc.tile_pool(name="t1", bufs=2))
        v1_pool = ctx.enter_context(tc.tile_pool(name="v1", bufs=2))
        e1_pool = ctx.enter_context(tc.tile_pool(name="e1", bufs=2))
        t2_pool = ctx.enter_context(tc.tile_pool(name="t2", bufs=3))
        v2_pool = ctx.enter_context(tc.tile_pool(name="v2", bufs=3))
        e2_pool = ctx.enter_context(tc.tile_pool(name="e2", bufs=2))
        small = ctx.enter_context(tc.tile_pool(name="small", bufs=2))

        wsb = {}
        for k, shp in w_specs.items():
            wt = wpool.tile(shp, f32, tag=f"w_{k}")
            nc.sync.dma_start(wt[:], w_dram[k][:])
            wsb[k] = wt
        corr_sb = wpool.tile([LO, nodes], f32, tag="corr")
        nc.sync.dma_start(corr_sb[:], corr_dram[:])
        mi_sb = wpool.tile([1, nodes], f32, tag="mi")
        nc.sync.dma_start(mi_sb[:], mi_dram[:])
        xi_sb = wpool.tile([3, nodes], f32, tag="xi")
        nc.sync.dma_start(xi_sb[:], xi_dram[:])

        if not use_fanin:
            atm_sb = wpool.tile([LO, nodes], f32, tag="atm")
        ztag = "za" if MERGE2 else "zb"

        def celu(z, tt, vv, out, t, layer, P):
            """out = celu(z + b) elementwise; z in PSUM, out in SBUF."""
            eb = wsb[f"eb{layer}{t}"] if t is not None else wsb[f"ebc{layer}"]
            rb = wsb[f"rb{layer}{t}"] if t is not None else wsb[f"rbc{layer}"]
            if abl != "noact":
                nc.scalar.activation(tt, z, Act.Exp, bias=eb[:P, :],
                                     scale=INV_ALPHA)
            if abl == "nodve":
                return
            ts_eng = nc.gpsimd if ts_gpsimd else nc.vector
            ts_eng.tensor_scalar(vv, tt, ALPHA, ALPHA, Alu.min, Alu.subtract)
            nc.vector.scalar_tensor_tensor(out, z, rb[:P, :], vv, Alu.add, Alu.max)

        rep_cm = tc.For_i(0, reps, 1) if reps > 1 else None
        if rep_cm is not None:
            ctx.enter_context(rep_cm)

        for p in range(nphase):
            nsl = slice(p * PH, (p + 1) * PH)      # node slice of this phase
            if use_fanin:
                trunk = tr_pool.tile([C1, PH], f32, tag="trunk")

            # ---- ai path (also initializes the trunk accumulation) ----
            zi = zb_pool.tile([LH, PH], f32, tag=ztag)
            nc.tensor.matmul(zi[:], wsb["w1i"][:], xi_sb[:, nsl],
                             start=True, stop=True)
            ti = t2_pool.tile([LH, PH], f32, tag="t2")
            vi = v2_pool.tile([LH, PH], f32, tag="v2")
            e1i = e1_pool.tile([LH, PH], f32, tag="e1")
            celu(zi[:], ti[:], vi[:], e1i[:], "i", 1, LH)

            zi2 = zb_pool.tile([LH, PH], f32, tag=ztag)
            nc.tensor.matmul(zi2[:], wsb["w2i"][:], e1i[:], start=True, stop=True)
            ti2 = t2_pool.tile([LH, PH], f32, tag="t2")
            vi2 = v2_pool.tile([LH, PH], f32, tag="v2")
            e2i = e2_pool.tile([LH, PH], f32, tag="e2")
            celu(zi2[:], ti2[:], vi2[:], e2i[:], "i", 2, LH)

            if use_fanin:
                nc.tensor.matmul(trunk[:], wsb["wc1"][:], e2i[:],
                                 start=True, stop=False, skip_group_check=True)
                nc.tensor.matmul(trunk[:], wsb["wc1"][:], corr_sb[:, nsl],
                                 start=False, stop=False, skip_group_check=True)
            else:
                nc.vector.tensor_copy(atm_sb[:, nsl], e2i[:])
                nc.vector.tensor_add(atm_sb[:, nsl], atm_sb[:, nsl],
                                     corr_sb[:, nsl])

            # ---- message streams j,k,l interleaved ----
            for g in range(ngrp):
                for t in "jkl":
                    off = p * rows_ph + g * G
                    xg = xpool.tile([3, G], f32, tag="xg")
                    if abl != "nodma":
                        nc.sync.dma_start(xg[:], x_dram[t][:, off:off + G])

                    za = za_pool.tile([LH, G], f32, tag="za")
                    for h in range(2):
                        cs = slice(h * 512, (h + 1) * 512)
                        nc.tensor.matmul(za[:, cs], wsb[f"w1{t}"][:], xg[:, cs],
                                         start=True, stop=True)
                    t1 = t1_pool.tile([LH, G], f32, tag="t1")
                    v1 = v1_pool.tile([LH, G], f32, tag="v1")
                    if SPLIT1:
                        # celu(z+b) = Relu(z+b) + (min(a*e^((z+b)/a), a) - a);
                        # feed both addends through L2 (accumulating matmuls)
                        r1 = e1_pool.tile([LH, G], f32, tag="e1")
                        if abl != "noact":
                            nc.scalar.activation(r1[:], za[:], Act.Relu,
                                                 bias=wsb[f"rb1{t}"][:],
                                                 scale=1.0)
                            nc.scalar.activation(t1[:], za[:], Act.Exp,
                                                 bias=wsb[f"eb1{t}"][:],
                                                 scale=INV_ALPHA)
                        if abl != "nodve":
                            nc.vector.tensor_scalar(v1[:], t1[:], ALPHA, ALPHA,
                                                    Alu.min, Alu.subtract)
                        e1 = None
                    else:
                        e1 = e1_pool.tile([LH, G], f32, tag="e1")
                        celu(za[:], t1[:], v1[:], e1[:], t, 1, LH)

                    e2 = e2_pool.tile([LO, G], f32, tag="e2")
                    mmaj = use_fanin and not STRIDED_RHS
                    if mmaj:
                        # m-major layout: addr = m*npg + n
                        e2r = e2[:].rearrange("p (m n) -> p n m", m=M)
                    else:
                        e2r = e2[:].rearrange("p (n m) -> p n m", m=M)
                    if MERGE2:
                        zb = zb_pool.tile([LO, G], f32, tag="za")
                        for h in range(2):
                            cs = slice(h * 512, (h + 1) * 512)
                            if SPLIT1:
                                nc.tensor.matmul(zb[:, cs], wsb[f"w2{t}"][:],
                                                 r1[:, cs], start=True,
                                                 stop=False)
                                nc.tensor.matmul(zb[:, cs], wsb[f"w2{t}"][:],
                                                 v1[:, cs], start=False,
                                                 stop=True)
                            else:
                                nc.tensor.matmul(zb[:, cs], wsb[f"w2{t}"][:],
                                                 e1[:, cs], start=True,
                                                 stop=True)
                        t2 = t2_pool.tile([LO, G], f32, tag="t2")
                        v2 = v2_pool.tile([LO, G], f32, tag="v2")
                        ov = e2r if mmaj else e2[:]
                        if abl != "noact":
                            nc.scalar.activation(t2[:], zb[:], Act.Exp,
                                                 bias=wsb[f"eb2{t}"][:],
                                                 scale=INV_ALPHA)
                        if abl != "nodve":
                            nc.vector.tensor_scalar(v2[:], t2[:], ALPHA, ALPHA,
                                                    Alu.min, Alu.subtract)
                            nc.vector.scalar_tensor_tensor(ov, zb[:],
                                                           wsb[f"rb2{t}"][:],
                                                           v2[:], Alu.add,
                                                           Alu.max)
                    else:
                        for h in range(2):
                            cs = slice(h * 512, (h + 1) * 512)
                            zb = zb_pool.tile([LO, 512], f32, tag="zb")
                            if SPLIT1:
                                nc.tensor.matmul(zb[:], wsb[f"w2{t}"][:],
                                                 r1[:, cs], start=True,
                                                 stop=False)
                                nc.tensor.matmul(zb[:], wsb[f"w2{t}"][:],
                                                 v1[:, cs], start=False,
                                                 stop=True)
                            else:
                                nc.tensor.matmul(zb[:], wsb[f"w2{t}"][:],
                                                 e1[:, cs], start=True,
                                                 stop=True)
                            t2 = t2_pool.tile([LO, 512], f32, tag="t2")
                            v2 = v2_pool.tile([LO, 512], f32, tag="v2")
                            if mmaj:
                                ov = e2r[:, h * (npg // 2):(h + 1) * (npg // 2), :]
                            else:
                                ov = e2[:, cs]
                            eb = wsb[f"eb2{t}"]
                            rb = wsb[f"rb2{t}"]
                            if abl != "noact":
                                nc.scalar.activation(t2[:], zb[:], Act.Exp,
                                                     bias=eb[:], scale=INV_ALPHA)
                            if abl != "nodve":
                                ts_eng = nc.gpsimd if ts_gpsimd else nc.vector
                                ts_eng.tensor_scalar(v2[:], t2[:], ALPHA, ALPHA,
                                                     Alu.min, Alu.subtract)
                                nc.vector.scalar_tensor_tensor(ov, zb[:], rb[:],
                                                               v2[:], Alu.add,
                                                               Alu.max)

                    if use_fanin:
                        # accumulate sum_m (wc1.T @ e2[:, (n,m)]) into
                        # trunk[:, g*npg : (g+1)*npg] via repeated out AP
                        tv = trunk[:, g * npg:(g + 1) * npg]
                        fan_ap = bass.AP(tensor=tv.tensor, offset=tv.offset,
                                         ap=[list(tv.ap[0]), [0, M // 2],
                                             list(tv.ap[-1])])
                        if STRIDED_RHS:
                            # iterate m outer / n inner over the plain layout
                            e2mn = e2[:].rearrange("p (n m) -> p m n", m=M)
                        for mh in range(2):
                            if STRIDED_RHS:
                                rhs = e2mn[:, mh * (M // 2):(mh + 1) * (M // 2), :]
                            else:
                                rhs = e2[:, mh * 512:(mh + 1) * 512]
                            nc.tensor.matmul(fan_ap, wsb["wc1"][:], rhs,
                                             start=False, stop=False,
                                             skip_group_check=True)
                    else:
                        red = small.tile([LO, npg], f32, tag="red")
                        nc.vector.tensor_reduce(red[:], e2r,
                                                axis=mybir.AxisListType.X,
                                                op=Alu.add)
                        asl = atm_sb[:, p * PH + g * npg:p * PH + (g + 1) * npg]
                        nc.vector.tensor_add(asl, asl, red[:])

            # ---- trunk chain for this phase ----
            if use_fanin:
                z1c = trunk[:]
            else:
                z1c_ps = tr_pool.tile([C1, PH], f32, tag="trunk")
                for h in range(PH // 512 if PH > 512 else 1):
                    pass
                nc.tensor.matmul(z1c_ps[:], wsb["wc1"][:], atm_sb[:, nsl],
                                 start=True, stop=True)
                z1c = z1c_ps[:]

            tc1 = t2_pool.tile([C1, PH], f32, tag="t2")
            vc1 = v2_pool.tile([C1, PH], f32, tag="v2")
            ec1 = e1_pool.tile([C1, PH], f32, tag="e1")
            celu(z1c, tc1[:], vc1[:], ec1[:], None, 1, C1)

            z2c = zb_pool.tile([C2, PH], f32, tag=ztag)
            nc.tensor.matmul(z2c[:], wsb["wc2"][:], ec1[:], start=True, stop=True)
            tc2 = t2_pool.tile([C2, PH], f32, tag="t2")
            vc2 = v2_pool.tile([C2, PH], f32, tag="v2")
            ec2 = e1_pool.tile([C2, PH], f32, tag="e1")
            celu(z2c[:], tc2[:], vc2[:], ec2[:], None, 2, C2)

            z3c = zb_pool.tile([C3, PH], f32, tag=ztag)
            nc.tensor.matmul(z3c[:], wsb["wc3"][:], ec2[:], start=True, stop=True)
            tc3 = t2_pool.tile([C3, PH], f32, tag="t2")
            vc3 = v2_pool.tile([C3, PH], f32, tag="v2")
            ec3 = e1_pool.tile([C3, PH], f32, tag="e1")
            celu(z3c[:], tc3[:], vc3[:], ec3[:], None, 3, C3)

            z4c = zb_pool.tile([1, PH], f32, tag=ztag)
            nc.tensor.matmul(z4c[:], wsb["wc4"][:], ec3[:], start=True, stop=True)
            o = small.tile([1, PH], f32, tag="o")
            nc.scalar.activation(o[:], z4c[:], Act.Identity,
                                 bias=wsb["bc4"][:], scale=1.0)
            om = small.tile([1, PH], f32, tag="om")
            nc.vector.tensor_mul(om[:], o[:], mi_sb[:, nsl])
            nc.sync.dma_start(out_dram[:, nsl], om[:])

    nc.compile()
    return nc


# --------------------------------------------------------------------------
# host side
# --------------------------------------------------------------------------

def _celu_np(x):
    x = x.astype(np.float32)
    return (np.maximum(x, 0.0)
            + np.minimum(0.0, np.float32(ALPHA)
                         * np.expm1(x * np.float32(INV_ALPHA)))).astype(np.float32)


def _prep_core(inputs, c, nmol=BL):
    s = slice(c * nmol, (c + 1) * nmol)
    nodes = nmol * N
    d = {}
    for t, key in (("j", "aj"), ("k", "ak"), ("l", "al")):
        flat = np.ascontiguousarray(inputs[key][s], dtype=np.float32).reshape(-1, 3)
        d[f"x{t}"] = np.ascontiguousarray(flat.T)
    ai = np.ascontiguousarray(inputs["ai"][s], dtype=np.float32).reshape(-1, 3)
    d["xi"] = np.ascontiguousarray(ai.T)
    mi = ((ai[:, 0] + ai[:, 1]) + ai[:, 2]) != 0
    d["mi"] = mi.astype(np.float32)[None, :]

    corr = np.zeros((nodes, LO), np.float32)
    for key, wn in (("aj", "j"), ("ak", "k"), ("al", "l")):
        flat = np.ascontiguousarray(inputs[key][s], dtype=np.float32).reshape(-1, 3)
        ssum = (flat[:, 0] + flat[:, 1]) + flat[:, 2]
        idx = np.nonzero(ssum == 0)[0]
        if idx.size:
            W1 = inputs[f"W{wn}1"].astype(np.float32)
            b1 = inputs[f"b{wn}1"].astype(np.float32)
            W2 = inputs[f"W{wn}2"].astype(np.float32)
            b2 = inputs[f"b{wn}2"].astype(np.float32)
            h1 = _celu_np(flat[idx] @ W1.T + b1)
            h2 = _celu_np(h1 @ W2.T + b2)
            np.subtract.at(corr, idx // M, h2)
    d["corr"] = np.ascontiguousarray(corr.T)

    def ct(a):
        return np.ascontiguousarray(a, dtype=np.float32)

    for t, wn in (("j", "j"), ("k", "k"), ("l", "l"), ("i", "i")):
        W1 = inputs[f"W{wn}1"]
        b1 = inputs[f"b{wn}1"].astype(np.float32)
        W2 = inputs[f"W{wn}2"]
        b2 = inputs[f"b{wn}2"].astype(np.float32)
        d[f"w1{t}"] = ct(W1.T)
        d[f"w2{t}"] = ct(W2.T)
        d[f"eb1{t}"] = ct(b1 * INV_ALPHA + LN_ALPHA)[:, None]
        d[f"rb1{t}"] = ct(b1)[:, None]
        d[f"eb2{t}"] = ct(b2 * INV_ALPHA + LN_ALPHA)[:, None]
        d[f"rb2{t}"] = ct(b2)[:, None]
    for i, (wk, bk) in enumerate((("W1", "b1"), ("W2", "b2"),
                                  ("W3", "b3"), ("W4", "b4")), start=1):
        W = inputs[wk]
        bb = inputs[bk].astype(np.float32)
        d[f"wc{i}"] = ct(W.T)
        if i < 4:
            d[f"ebc{i}"] = ct(bb * INV_ALPHA + LN_ALPHA)[:, None]
            d[f"rbc{i}"] = ct(bb)[:, None]
        else:
            d["bc4"] = ct(bb)[:, None]
    return d


def _get_program(nmol=BL):
    key = (nmol, USE_FANIN)
    if key not in _PROGRAM_CACHE:
        _PROGRAM_CACHE[key] = _build_program(nmol=nmol, use_fanin=USE_FANIN)
    return _PROGRAM_CACHE[key]


def run(inputs, trace=False, **kwargs):
    """Returns (full_output [B,N,1] f32, BassKernelResults)."""
    from concourse.bass_utils import run_bass_kernel_spmd
    inputs = {k: np.asarray(v) for k, v in inputs.items()}
    nc = _get_program()
    in_maps = [_prep_core(inputs, c) for c in range(NCORES)]
    res = run_bass_kernel_spmd(nc, in_maps, core_ids=list(range(NCORES)),
                               trace=trace, **kwargs)
    outs = [res.results[c]["out"].reshape(BL, N, 1) for c in range(NCORES)]
    return np.concatenate(outs, axis=0).astype(np.float32), res


def kernel(**inputs):
    out, _ = run(inputs)
    return out


# revision 24
# speedup vs baseline: 1.1284x; 1.1284x over previous
"""Trainium2 Bass kernel for nn_DART_Net (gnn_message_passing).

Math (per molecule b, node n):
  hi = mlp2_i(ai) * mask(ai)                 [B,N,128]
  h{j,k,l} = mlp2_t(a_t) * mask(a_t)        [B,N,M,128] -> sum over M
  atm = hi + sum_j + sum_k + sum_l
  out = (celu-chain(atm) @ W4.T + b4) * mask(ai)
with mlp2(x) = celu(celu(x@W1.T+b1)@W2.T+b2), celu alpha=0.1.

Device strategy (per core, pure data parallel over B):
  - features on partitions, message rows on the free axis
  - celu(z+b) == max(z+b, min(alpha*e^((z+b)/alpha), alpha) - alpha)
      ACT:  t = Exp(z/alpha + (b/alpha + ln(alpha)))   (= alpha*e^((z+b)/alpha))
      DVE:  v = (t min alpha) sub alpha                (tensor_scalar dual-op)
      DVE:  e = (z add b) max v                        (scalar_tensor_tensor)
  - sum over M: trunk layer-1 is linear, so W1c @ sum_m e2 == sum_m W1c @ e2;
    accumulate straight into a phase-resident PSUM bank with a repeated
    (step-0) matmul output access pattern (fan-in).  Fallback: DVE reduce.
  - pad masks: ai mask applied at the output on device; exact-zero message
    rows (probability ~0 with randn inputs) are fixed up via a host-computed
    per-node additive correction "corr" that enters the same trunk matmul.
"""

import sys
import numpy as np
from contextlib import ExitStack

for _p in ("/opt/trn_rl_repo", "/root/.axon_site/_ro/trn_rl_repo"):
    if _p not in sys.path:
        sys.path.append(_p)

ALPHA = 0.1
INV_ALPHA = 1.0 / ALPHA
LN_ALPHA = float(np.log(np.float32(ALPHA)))

B, N, M = 64, 128, 64
NCORES = 8
BL = B // NCORES          # molecules per core
LH, LO = 128, 128
C1, C2, C3 = 64, 32, 16

USE_FANIN = True         # PE fan-in m-reduce; False -> DVE tensor_reduce
TS_ON_GPSIMD = False     # run the tensor_scalar (v = min(t,a)-a) on GPSIMD
SPLIT1 = False           # celu1 = Relu(z+b) + v via two accumulating L2 matmuls
MERGE2 = False            # single [128,1024] L2 psum; one exp2/TS2/STT2 per group
STRIDED_RHS = True       # fan-in matmul reads e2 strided; e2 written contiguous

_PROGRAM_CACHE = {}


# --------------------------------------------------------------------------
# device program
# --------------------------------------------------------------------------

def _build_program(nmol=BL, debug=False, use_fanin=USE_FANIN, reps=1,
                   ts_gpsimd=None, abl=None):
    # abl: None | "nodve" | "noact" | "nodma"  (timing ablations, wrong math)
    if ts_gpsimd is None:
        ts_gpsimd = TS_ON_GPSIMD
    import concourse.bass as bass
    import concourse.tile as tile
    from concourse import bacc, mybir

    f32 = mybir.dt.float32
    Alu = mybir.AluOpType
    Act = mybir.ActivationFunctionType

    nodes = nmol * N                  # nodes per core
    rmsg = nodes * M                  # message rows per tensor per core
    PH = min(512, nodes)              # nodes per trunk phase (1 PSUM bank)
    nphase = nodes // PH
    G = 1024                          # message columns per group
    rows_ph = PH * M                  # message rows per phase per tensor
    ngrp = rows_ph // G
    npg = G // M                      # nodes per group (16)

    nc = bacc.Bacc("TRN2", target_bir_lowering=False, debug=debug)

    x_dram = {t: nc.dram_tensor(f"x{t}", [3, rmsg], f32, kind="ExternalInput")
              for t in "jkl"}
    xi_dram = nc.dram_tensor("xi", [3, nodes], f32, kind="ExternalInput")
    corr_dram = nc.dram_tensor("corr", [LO, nodes], f32, kind="ExternalInput")
    mi_dram = nc.dram_tensor("mi", [1, nodes], f32, kind="ExternalInput")

    w_specs = {}
    for t in "jkli":
        w_specs[f"w1{t}"] = [3, LH]
        w_specs[f"w2{t}"] = [LH, LO]
        w_specs[f"eb1{t}"] = [LH, 1]
        w_specs[f"rb1{t}"] = [LH, 1]
        w_specs[f"eb2{t}"] = [LO, 1]
        w_specs[f"rb2{t}"] = [LO, 1]
    w_specs.update(wc1=[LO, C1], ebc1=[C1, 1], rbc1=[C1, 1],
                   wc2=[C1, C2], ebc2=[C2, 1], rbc2=[C2, 1],
                   wc3=[C2, C3], ebc3=[C3, 1], rbc3=[C3, 1],
                   wc4=[C3, 1], bc4=[1, 1])
    w_dram = {k: nc.dram_tensor(k, v, f32, kind="ExternalInput")
              for k, v in w_specs.items()}
    out_dram = nc.dram_tensor("out", [1, nodes], f32, kind="ExternalOutput")

    with ExitStack() as ctx:
        tc = ctx.enter_context(tile.TileContext(nc))

        wpool = ctx.enter_context(tc.tile_pool(name="w", bufs=1))
        xpool = ctx.enter_context(tc.tile_pool(name="x", bufs=4))
        if MERGE2:
            # one shared psum pool: [128,1024] tiles (2 banks) x 3 bufs
            z_pool = ctx.enter_context(tc.tile_pool(name="z", bufs=3,
                                                    space="PSUM"))
            za_pool = zb_pool = z_pool
        else:
            za_pool = ctx.enter_context(tc.tile_pool(name="za", bufs=2,
                                                     space="PSUM"))
            zb_pool = ctx.enter_context(tc.tile_pool(name="zb", bufs=3,
                                                     space="PSUM"))
        tr_pool = ctx.enter_context(tc.tile_pool(name="tr", bufs=1, space="PSUM"))
        t1_pool = ctx.enter_context(tc.tile_pool(name="t1", bufs=2))
        v1_pool = ctx.enter_context(tc.tile_pool(name="v1", bufs=2))
        e1_pool = ctx.enter_context(tc.tile_pool(name="e1", bufs=2))
        t2_pool = ctx.enter_context(tc.tile_pool(name="t2", bufs=3))
        v2_pool = ctx.enter_context(tc.tile_pool(name="v2", bufs=3))
        e2_pool = ctx.enter_context(tc.tile_pool(name="e2", bufs=2))
        small = ctx.enter_context(tc.tile_pool(name="small", bufs=2))

        wsb = {}
        for k, shp in w_specs.items():
            wt = wpool.tile(shp, f32, tag=f"w_{k}")
            nc.sync.dma_start(wt[:], w_dram[k][:])
            wsb[k] = wt
        corr_sb = wpool.tile([LO, nodes], f32, tag="corr")
        nc.sync.dma_start(corr_sb[:], corr_dram[:])
        mi_sb = wpool.tile([1, nodes], f32, tag="mi")
        nc.sync.dma_start(mi_sb[:], mi_dram[:])
        xi_sb = wpool.tile([3, nodes], f32, tag="xi")
        nc.sync.dma_start(xi_sb[:], xi_dram[:])

        if not use_fanin:
            atm_sb = wpool.tile([LO, nodes], f32, tag="atm")
        ztag = "za" if MERGE2 else "zb"

        def celu(z, tt, vv, out, t, layer, P):
            """out = celu(z + b) elementwise; z in PSUM, out in SBUF."""
            eb = wsb[f"eb{layer}{t}"] if t is not None else wsb[f"ebc{layer}"]
            rb = wsb[f"rb{layer}{t}"] if t is not None else wsb[f"rbc{layer}"]
            if abl != "noact":
                nc.scalar.activation(tt, z, Act.Exp, bias=eb[:P, :],
                                     scale=INV_ALPHA)
            if abl == "nodve":
                return
            ts_eng = nc.gpsimd if ts_gpsimd else nc.vector
            ts_eng.tensor_scalar(vv, tt, ALPHA, ALPHA, Alu.min, Alu.subtract)
            nc.vector.scalar_tensor_tensor(out, z, rb[:P, :], vv, Alu.add, Alu.max)

        rep_cm = tc.For_i(0, reps, 1) if reps > 1 else None
        if rep_cm is not None:
            ctx.enter_context(rep_cm)

        for p in range(nphase):
            nsl = slice(p * PH, (p + 1) * PH)      # node slice of this phase
            if use_fanin:
                trunk = tr_pool.tile([C1, PH], f32, tag="trunk")

            # ---- ai path (also initializes the trunk accumulation) ----
            zi = zb_pool.tile([LH, PH], f32, tag=ztag)
            nc.tensor.matmul(zi[:], wsb["w1i"][:], xi_sb[:, nsl],
                             start=True, stop=True)
            ti = t2_pool.tile([LH, PH], f32, tag="t2")
            vi = v2_pool.tile([LH, PH], f32, tag="v2")
            e1i = e1_pool.tile([LH, PH], f32, tag="e1")
            celu(zi[:], ti[:], vi[:], e1i[:], "i", 1, LH)

            zi2 = zb_pool.tile([LH, PH], f32, tag=ztag)
            nc.tensor.matmul(zi2[:], wsb["w2i"][:], e1i[:], start=True, stop=True)
            ti2 = t2_pool.tile([LH, PH], f32, tag="t2")
            vi2 = v2_pool.tile([LH, PH], f32, tag="v2")
            e2i = e2_pool.tile([LH, PH], f32, tag="e2")
            celu(zi2[:], ti2[:], vi2[:], e2i[:], "i", 2, LH)

            if use_fanin:
                nc.tensor.matmul(trunk[:], wsb["wc1"][:], e2i[:],
                                 start=True, stop=False, skip_group_check=True)
                nc.tensor.matmul(trunk[:], wsb["wc1"][:], corr_sb[:, nsl],
                                 start=False, stop=False, skip_group_check=True)
            else:
                nc.vector.tensor_copy(atm_sb[:, nsl], e2i[:])
                nc.vector.tensor_add(atm_sb[:, nsl], atm_sb[:, nsl],
                                     corr_sb[:, nsl])

            # ---- message streams j,k,l interleaved ----
            for g in range(ngrp):
                for t in "jkl":
                    off = p * rows_ph + g * G
                    xg = xpool.tile([3, G], f32, tag="xg")
                    if abl != "nodma":
                        nc.sync.dma_start(xg[:], x_dram[t][:, off:off + G])

                    za = za_pool.tile([LH, G], f32, tag="za")
                    for h in range(2):
                        cs = slice(h * 512, (h + 1) * 512)
                        nc.tensor.matmul(za[:, cs], wsb[f"w1{t}"][:], xg[:, cs],
                                         start=True, stop=True)
                    t1 = t1_pool.tile([LH, G], f32, tag="t1")
                    v1 = v1_pool.tile([LH, G], f32, tag="v1")
                    if SPLIT1:
                        # celu(z+b) = Relu(z+b) + (min(a*e^((z+b)/a), a) - a);
                        # feed both addends through L2 (accumulating matmuls)
                        r1 = e1_pool.tile([LH, G], f32, tag="e1")
                        if abl != "noact":
                            nc.scalar.activation(r1[:], za[:], Act.Relu,
                                                 bias=wsb[f"rb1{t}"][:],
                                                 scale=1.0)
                            nc.scalar.activation(t1[:], za[:], Act.Exp,
                                                 bias=wsb[f"eb1{t}"][:],
                                                 scale=INV_ALPHA)
                        if abl != "nodve":
                            nc.vector.tensor_scalar(v1[:], t1[:], ALPHA, ALPHA,
                                                    Alu.min, Alu.subtract)
                        e1 = None
                    else:
                        e1 = e1_pool.tile([LH, G], f32, tag="e1")
                        celu(za[:], t1[:], v1[:], e1[:], t, 1, LH)

                    e2 = e2_pool.tile([LO, G], f32, tag="e2")
                    mmaj = use_fanin and not STRIDED_RHS
                    if mmaj:
                        # m-major layout: addr = m*npg + n
                        e2r = e2[:].rearrange("p (m n) -> p n m", m=M)
                    else:
                        e2r = e2[:].rearrange("p (n m) -> p n m", m=M)
                    if MERGE2:
                        zb = zb_pool.tile([LO, G], f32, tag="za")
                        for h in range(2):
                            cs = slice(h * 512, (h + 1) * 512)
                            if SPLIT1:
                                nc.tensor.matmul(zb[:, cs], wsb[f"w2{t}"][:],
                                                 r1[:, cs], start=True,
                                                 stop=False)
                                nc.tensor.matmul(zb[:, cs], wsb[f"w2{t}"][:],
                                                 v1[:, cs], start=False,
                                                 stop=True)
                            else:
                                nc.tensor.matmul(zb[:, cs], wsb[f"w2{t}"][:],
                                                 e1[:, cs], start=True,
                                                 stop=True)
                        t2 = t2_pool.tile([LO, G], f32, tag="t2")
                        v2 = v2_pool.tile([LO, G], f32, tag="v2")
                        ov = e2r if mmaj else e2[:]
                        if abl != "noact":
                            nc.scalar.activation(t2[:], zb[:], Act.Exp,
                                                 bias=wsb[f"eb2{t}"][:],
                                                 scale=INV_ALPHA)
                        if abl != "nodve":
                            nc.vector.tensor_scalar(v2[:], t2[:], ALPHA, ALPHA,
                                                    Alu.min, Alu.subtract)
                            nc.vector.scalar_tensor_tensor(ov, zb[:],
                                                           wsb[f"rb2{t}"][:],
                                                           v2[:], Alu.add,
                                                           Alu.max)
                    else:
                        for h in range(2):
                            cs = slice(h * 512, (h + 1) * 512)
                            zb = zb_pool.tile([LO, 512], f32, tag="zb")
                            if SPLIT1:
                                nc.tensor.matmul(zb[:], wsb[f"w2{t}"][:],
                                                 r1[:, cs], start=True,
                                                 stop=False)
                                nc.tensor.matmul(zb[:], wsb[f"w2{t}"][:],
                                                 v1[:, cs], start=False,
                                                 stop=True)
                            else:
                                nc.tensor.matmul(zb[:], wsb[f"w2{t}"][:],
                                                 e1[:, cs], start=True,
                                                 stop=True)
                            t2 = t2_pool.tile([LO, 512], f32, tag="t2")
                            v2 = v2_pool.tile([LO, 512], f32, tag="v2")
                            if mmaj:
                                ov = e2r[:, h * (npg // 2):(h + 1) * (npg // 2), :]
                            else:
                                ov = e2[:, cs]
                            eb = wsb[f"eb2{t}"]
                            rb = wsb[f"rb2{t}"]
                            if abl != "noact":
                                nc.scalar.activation(t2[:], zb[:], Act.Exp,
                                                     bias=eb[:], scale=INV_ALPHA)
                            if abl != "nodve":
                                ts_eng = nc.gpsimd if ts_gpsimd else nc.vector
                                ts_eng.tensor_scalar(v2[:], t2[:], ALPHA, ALPHA,
                                                     Alu.min, Alu.subtract)
                                nc.vector.scalar_tensor_tensor(ov, zb[:], rb[:],
                                                               v2[:], Alu.add,
                                                               Alu.max)

                    if use_fanin:
                        # accumulate sum_m (wc1.T @ e2[:, (n,m)]) into
                        # trunk[:, g*npg : (g+1)*npg] via repeated out AP
                        tv = trunk[:, g * npg:(g + 1) * npg]
                        fan_ap = bass.AP(tensor=tv.tensor, offset=tv.offset,
                                         ap=[list(tv.ap[0]), [0, M // 2],
                                             list(tv.ap[-1])])
                        if STRIDED_RHS:
                            # iterate m outer / n inner over the plain layout
                            e2mn = e2[:].rearrange("p (n m) -> p m n", m=M)
                        for mh in range(2):
                            if STRIDED_RHS:
                                rhs = e2mn[:, mh * (M // 2):(mh + 1) * (M // 2), :]
                            else:
                                rhs = e2[:, mh * 512:(mh + 1) * 512]
                            nc.tensor.matmul(fan_ap, wsb["wc1"][:], rhs,
                                             start=False, stop=False,
                                             skip_group_check=True)
                    else:
                        red = small.tile([LO, npg], f32, tag="red")
                        nc.vector.tensor_reduce(red[:], e2r,
                                                axis=mybir.AxisListType.X,
                                                op=Alu.add)
                        asl = atm_sb[:, p * PH + g * npg:p * PH + (g + 1) * npg]
                        nc.vector.tensor_add(asl, asl, red[:])

            # ---- trunk chain for this phase ----
            if use_fanin:
                z1c = trunk[:]
            else:
                z1c_ps = tr_pool.tile([C1, PH], f32, tag="trunk")
                for h in range(PH // 512 if PH > 512 else 1):
                    pass
                nc.tensor.matmul(z1c_ps[:], wsb["wc1"][:], atm_sb[:, nsl],
                                 start=True, stop=True)
                z1c = z1c_ps[:]

            tc1 = t2_pool.tile([C1, PH], f32, tag="t2")
            vc1 = v2_pool.tile([C1, PH], f32, tag="v2")
            ec1 = e1_pool.tile([C1, PH], f32, tag="e1")
            celu(z1c, tc1[:], vc1[:], ec1[:], None, 1, C1)

            z2c = zb_pool.tile([C2, PH], f32, tag=ztag)
            nc.tensor.matmul(z2c[:], wsb["wc2"][:], ec1[:], start=True, stop=True)
            tc2 = t2_pool.tile([C2, PH], f32, tag="t2")
            vc2 = v2_pool.tile([C2, PH], f32, tag="v2")
            ec2 = e1_pool.tile([C2, PH], f32, tag="e1")
            celu(z2c[:], tc2[:], vc2[:], ec2[:], None, 2, C2)

            z3c = zb_pool.tile([C3, PH], f32, tag=ztag)
            nc.tensor.matmul(z3c[:], wsb["wc3"][:], ec2[:], start=True, stop=True)
            tc3 = t2_pool.tile([C3, PH], f32, tag="t2")
            vc3 = v2_pool.tile([C3, PH], f32, tag="v2")
            ec3 = e1_pool.tile([C3, PH], f32, tag="e1")
            celu(z3c[:], tc3[:], vc3[:], ec3[:], None, 3, C3)

            z4c = zb_pool.tile([1, PH], f32, tag=ztag)
            nc.tensor.matmul(z4c[:], wsb["wc4"][:], ec3[:], start=True, stop=True)
            o = small.tile([1, PH], f32, tag="o")
            nc.scalar.activation(o[:], z4c[:], Act.Identity,
                                 bias=wsb["bc4"][:], scale=1.0)
            om = small.tile([1, PH], f32, tag="om")
            nc.vector.tensor_mul(om[:], o[:], mi_sb[:, nsl])
            nc.sync.dma_start(out_dram[:, nsl], om[:])

    nc.compile()
    return nc


# --------------------------------------------------------------------------
# host side
# --------------------------------------------------------------------------

def _celu_np(x):
    x = x.astype(np.float32)
    return (np.maximum(x, 0.0)
            + np.minimum(0.0, np.float32(ALPHA)
                         * np.expm1(x * np.float32(INV_ALPHA)))).astype(np.float32)


def _prep_core(inputs, c, nmol=BL):
    s = slice(c * nmol, (c + 1) * nmol)
    nodes = nmol * N
    d = {}
    for t, key in (("j", "aj"), ("k", "ak"), ("l", "al")):
        flat = np.ascontiguousarray(inputs[key][s], dtype=np.float32).reshape(-1, 3)
        d[f"x{t}"] = np.ascontiguousarray(flat.T)
    ai = np.ascontiguousarray(inputs["ai"][s], dtype=np.float32).reshape(-1, 3)
    d["xi"] = np.ascontiguousarray(ai.T)
    mi = ((ai[:, 0] + ai[:, 1]) + ai[:, 2]) != 0
    d["mi"] = mi.astype(np.float32)[None, :]

    corr = np.zeros((nodes, LO), np.float32)
    for key, wn in (("aj", "j"), ("ak", "k"), ("al", "l")):
        flat = np.ascontiguousarray(inputs[key][s], dtype=np.float32).reshape(-1, 3)
        ssum = (flat[:, 0] + flat[:, 1]) + flat[:, 2]
        idx = np.nonzero(ssum == 0)[0]
        if idx.size:
            W1 = inputs[f"W{wn}1"].astype(np.float32)
            b1 = inputs[f"b{wn}1"].astype(np.float32)
            W2 = inputs[f"W{wn}2"].astype(np.float32)
            b2 = inputs[f"b{wn}2"].astype(np.float32)
            h1 = _celu_np(flat[idx] @ W1.T + b1)
            h2 = _celu_np(h1 @ W2.T + b2)
            np.subtract.at(corr, idx // M, h2)
    d["corr"] = np.ascontiguousarray(corr.T)

    def ct(a):
        return np.ascontiguousarray(a, dtype=np.float32)

    for t, wn in (("j", "j"), ("k", "k"), ("l", "l"), ("i", "i")):
        W1 = inputs[f"W{wn}1"]
        b1 = inputs[f"b{wn}1"].astype(np.float32)
        W2 = inputs[f"W{wn}2"]
        b2 = inputs[f"b{wn}2"].astype(np.float32)
        d[f"w1{t}"] = ct(W1.T)
        d[f"w2{t}"] = ct(W2.T)
        d[f"eb1{t}"] = ct(b1 * INV_ALPHA + LN_ALPHA)[:, None]
        d[f"rb1{t}"] = ct(b1)[:, None]
        d[f"eb2{t}"] = ct(b2 * INV_ALPHA + LN_ALPHA)[:, None]
        d[f"rb2{t}"] = ct(b2)[:, None]
    for i, (wk, bk) in enumerate((("W1", "b1"), ("W2", "b2"),
                                  ("W3", "b3"), ("W4", "b4")), start=1):
        W = inputs[wk]
        bb = inputs[bk].astype(np.float32)
        d[f"wc{i}"] = ct(W.T)
        if i < 4:
            d[f"ebc{i}"] = ct(bb * INV_ALPHA + LN_ALPHA)[:, None]
            d[f"rbc{i}"] = ct(bb)[:, None]
        else:
            d["bc4"] = ct(bb)[:, None]
    return d


def _get_program(nmol=BL):
    key = (nmol, USE_FANIN)
    if key not in _PROGRAM_CACHE:
        _PROGRAM_CACHE[key] = _build_program(nmol=nmol, use_fanin=USE_FANIN)
    return _PROGRAM_CACHE[key]


def run(inputs, trace=False, **kwargs):
    """Returns (full_output [B,N,1] f32, BassKernelResults)."""
    from concourse.bass_utils import run_bass_kernel_spmd
    inputs = {k: np.asarray(v) for k, v in inputs.items()}
    nc = _get_program()
    in_maps = [_prep_core(inputs, c) for c in range(NCORES)]
    res = run_bass_kernel_spmd(nc, in_maps, core_ids=list(range(NCORES)),
                               trace=trace, **kwargs)
    outs = [res.results[c]["out"].reshape(BL, N, 1) for c in range(NCORES)]
    return np.concatenate(outs, axis=0).astype(np.float32), res


def kernel(**inputs):
    out, _ = run(inputs)
    return out


# revision 25
# speedup vs baseline: 1.2659x; 1.1218x over previous
"""Trainium2 Bass kernel for nn_DART_Net (gnn_message_passing).

Math (per molecule b, node n):
  hi = mlp2_i(ai) * mask(ai)                 [B,N,128]
  h{j,k,l} = mlp2_t(a_t) * mask(a_t)        [B,N,M,128] -> sum over M
  atm = hi + sum_j + sum_k + sum_l
  out = (celu-chain(atm) @ W4.T + b4) * mask(ai)
with mlp2(x) = celu(celu(x@W1.T+b1)@W2.T+b2), celu alpha=0.1.

Device strategy (per core, pure data parallel over B):
  - features on partitions, message rows on the free axis
  - celu(z+b) == max(z+b, min(alpha*e^((z+b)/alpha), alpha) - alpha)
      ACT:  t = Exp(z/alpha + (b/alpha + ln(alpha)))   (= alpha*e^((z+b)/alpha))
      DVE:  v = (t min alpha) sub alpha                (tensor_scalar dual-op)
      DVE:  e = (z add b) max v                        (scalar_tensor_tensor)
  - sum over M: trunk layer-1 is linear, so W1c @ sum_m e2 == sum_m W1c @ e2;
    accumulate straight into a phase-resident PSUM bank with a repeated
    (step-0) matmul output access pattern (fan-in).  Fallback: DVE reduce.
  - pad masks: ai mask applied at the output on device; exact-zero message
    rows (probability ~0 with randn inputs) are fixed up via a host-computed
    per-node additive correction "corr" that enters the same trunk matmul.
"""

import sys
import numpy as np
from contextlib import ExitStack

for _p in ("/opt/trn_rl_repo", "/root/.axon_site/_ro/trn_rl_repo"):
    if _p not in sys.path:
        sys.path.append(_p)

ALPHA = 0.1
INV_ALPHA = 1.0 / ALPHA
LN_ALPHA = float(np.log(np.float32(ALPHA)))

B, N, M = 64, 128, 64
NCORES = 8
BL = B // NCORES          # molecules per core
LH, LO = 128, 128
C1, C2, C3 = 64, 32, 16

USE_FANIN = True         # PE fan-in m-reduce; False -> DVE tensor_reduce
TS_ON_GPSIMD = False     # run the tensor_scalar (v = min(t,a)-a) on GPSIMD
SPLIT1 = False           # celu1 = Relu(z+b) + v via two accumulating L2 matmuls
MERGE2 = False            # single [128,1024] L2 psum; one exp2/TS2/STT2 per group
STRIDED_RHS = False       # fan-in matmul reads e2 strided; e2 written contiguous

_PROGRAM_CACHE = {}


# --------------------------------------------------------------------------
# device program
# --------------------------------------------------------------------------

def _build_program(nmol=BL, debug=False, use_fanin=USE_FANIN, reps=1,
                   ts_gpsimd=None, abl=None):
    # abl: None | "nodve" | "noact" | "nodma"  (timing ablations, wrong math)
    if ts_gpsimd is None:
        ts_gpsimd = TS_ON_GPSIMD
    import concourse.bass as bass
    import concourse.tile as tile
    from concourse import bacc, mybir

    f32 = mybir.dt.float32
    Alu = mybir.AluOpType
    Act = mybir.ActivationFunctionType

    nodes = nmol * N                  # nodes per core
    rmsg = nodes * M                  # message rows per tensor per core
    PH = min(512, nodes)              # nodes per trunk phase (1 PSUM bank)
    nphase = nodes // PH
    G = 1024                          # message columns per group
    rows_ph = PH * M                  # message rows per phase per tensor
    ngrp = rows_ph // G
    npg = G // M                      # nodes per group (16)

    nc = bacc.Bacc("TRN2", target_bir_lowering=False, debug=debug)

    x_dram = {t: nc.dram_tensor(f"x{t}", [3, rmsg], f32, kind="ExternalInput")
              for t in "jkl"}
    xi_dram = nc.dram_tensor("xi", [3, nodes], f32, kind="ExternalInput")
    corr_dram = nc.dram_tensor("corr", [LO, nodes], f32, kind="ExternalInput")
    mi_dram = nc.dram_tensor("mi", [1, nodes], f32, kind="ExternalInput")

    w_specs = {}
    for t in "jkli":
        w_specs[f"w1{t}"] = [3, LH]
        w_specs[f"w2{t}"] = [LH, LO]
        w_specs[f"eb1{t}"] = [LH, 1]
        w_specs[f"rb1{t}"] = [LH, 1]
        w_specs[f"eb2{t}"] = [LO, 1]
        w_specs[f"rb2{t}"] = [LO, 1]
    w_specs.update(wc1=[LO, C1], ebc1=[C1, 1], rbc1=[C1, 1],
                   wc2=[C1, C2], ebc2=[C2, 1], rbc2=[C2, 1],
                   wc3=[C2, C3], ebc3=[C3, 1], rbc3=[C3, 1],
                   wc4=[C3, 1], bc4=[1, 1])
    w_dram = {k: nc.dram_tensor(k, v, f32, kind="ExternalInput")
              for k, v in w_specs.items()}
    out_dram = nc.dram_tensor("out", [1, nodes], f32, kind="ExternalOutput")

    with ExitStack() as ctx:
        tc = ctx.enter_context(tile.TileContext(nc))

        wpool = ctx.enter_context(tc.tile_pool(name="w", bufs=1))
        xpool = ctx.enter_context(tc.tile_pool(name="x", bufs=6))
        if MERGE2:
            # one shared psum pool: [128,1024] tiles (2 banks) x 3 bufs
            z_pool = ctx.enter_context(tc.tile_pool(name="z", bufs=3,
                                                    space="PSUM"))
            za_pool = zb_pool = z_pool
        else:
            za_pool = ctx.enter_context(tc.tile_pool(name="za", bufs=2,
                                                     space="PSUM"))
            zb_pool = ctx.enter_context(tc.tile_pool(name="zb", bufs=3,
                                                     space="PSUM"))
        tr_pool = ctx.enter_context(tc.tile_pool(name="tr", bufs=1, space="PSUM"))
        t1_pool = ctx.enter_context(tc.tile_pool(name="t1", bufs=3))
        v1_pool = ctx.enter_context(tc.tile_pool(name="v1", bufs=3))
        e1_pool = ctx.enter_context(tc.tile_pool(name="e1", bufs=3))
        t2_pool = ctx.enter_context(tc.tile_pool(name="t2", bufs=5))
        v2_pool = ctx.enter_context(tc.tile_pool(name="v2", bufs=5))
        e2_pool = ctx.enter_context(tc.tile_pool(name="e2", bufs=3))
        small = ctx.enter_context(tc.tile_pool(name="small", bufs=2))

        wsb = {}
        for k, shp in w_specs.items():
            wt = wpool.tile(shp, f32, tag=f"w_{k}")
            nc.sync.dma_start(wt[:], w_dram[k][:])
            wsb[k] = wt
        corr_sb = wpool.tile([LO, nodes], f32, tag="corr")
        nc.sync.dma_start(corr_sb[:], corr_dram[:])
        mi_sb = wpool.tile([1, nodes], f32, tag="mi")
        nc.sync.dma_start(mi_sb[:], mi_dram[:])
        xi_sb = wpool.tile([3, nodes], f32, tag="xi")
        nc.sync.dma_start(xi_sb[:], xi_dram[:])

        if not use_fanin:
            atm_sb = wpool.tile([LO, nodes], f32, tag="atm")
        ztag = "za" if MERGE2 else "zb"

        def celu(z, tt, vv, out, t, layer, P):
            """out = celu(z + b) elementwise; z in PSUM, out in SBUF."""
            eb = wsb[f"eb{layer}{t}"] if t is not None else wsb[f"ebc{layer}"]
            rb = wsb[f"rb{layer}{t}"] if t is not None else wsb[f"rbc{layer}"]
            if abl != "noact":
                nc.scalar.activation(tt, z, Act.Exp, bias=eb[:P, :],
                                     scale=INV_ALPHA)
            if abl == "nodve":
                return
            ts_eng = nc.gpsimd if ts_gpsimd else nc.vector
            ts_eng.tensor_scalar(vv, tt, ALPHA, ALPHA, Alu.min, Alu.subtract)
            nc.vector.scalar_tensor_tensor(out, z, rb[:P, :], vv, Alu.add, Alu.max)

        rep_cm = tc.For_i(0, reps, 1) if reps > 1 else None
        if rep_cm is not None:
            ctx.enter_context(rep_cm)

        for p in range(nphase):
            nsl = slice(p * PH, (p + 1) * PH)      # node slice of this phase
            if use_fanin:
                trunk = tr_pool.tile([C1, PH], f32, tag="trunk")

            # ---- ai path (also initializes the trunk accumulation) ----
            zi = zb_pool.tile([LH, PH], f32, tag=ztag)
            nc.tensor.matmul(zi[:], wsb["w1i"][:], xi_sb[:, nsl],
                             start=True, stop=True)
            ti = t2_pool.tile([LH, PH], f32, tag="t2")
            vi = v2_pool.tile([LH, PH], f32, tag="v2")
            e1i = e1_pool.tile([LH, PH], f32, tag="e1")
            celu(zi[:], ti[:], vi[:], e1i[:], "i", 1, LH)

            zi2 = zb_pool.tile([LH, PH], f32, tag=ztag)
            nc.tensor.matmul(zi2[:], wsb["w2i"][:], e1i[:], start=True, stop=True)
            ti2 = t2_pool.tile([LH, PH], f32, tag="t2")
            vi2 = v2_pool.tile([LH, PH], f32, tag="v2")
            e2i = e2_pool.tile([LH, PH], f32, tag="e2")
            celu(zi2[:], ti2[:], vi2[:], e2i[:], "i", 2, LH)

            if use_fanin:
                nc.tensor.matmul(trunk[:], wsb["wc1"][:], e2i[:],
                                 start=True, stop=False, skip_group_check=True)
                nc.tensor.matmul(trunk[:], wsb["wc1"][:], corr_sb[:, nsl],
                                 start=False, stop=False, skip_group_check=True)
            else:
                nc.vector.tensor_copy(atm_sb[:, nsl], e2i[:])
                nc.vector.tensor_add(atm_sb[:, nsl], atm_sb[:, nsl],
                                     corr_sb[:, nsl])

            # ---- message streams j,k,l interleaved ----
            for g in range(ngrp):
                for t in "jkl":
                    off = p * rows_ph + g * G
                    xg = xpool.tile([3, G], f32, tag="xg")
                    if abl != "nodma":
                        nc.sync.dma_start(xg[:], x_dram[t][:, off:off + G])

                    za = za_pool.tile([LH, G], f32, tag="za")
                    for h in range(2):
                        cs = slice(h * 512, (h + 1) * 512)
                        nc.tensor.matmul(za[:, cs], wsb[f"w1{t}"][:], xg[:, cs],
                                         start=True, stop=True)
                    t1 = t1_pool.tile([LH, G], f32, tag="t1")
                    v1 = v1_pool.tile([LH, G], f32, tag="v1")
                    if SPLIT1:
                        # celu(z+b) = Relu(z+b) + (min(a*e^((z+b)/a), a) - a);
                        # feed both addends through L2 (accumulating matmuls)
                        r1 = e1_pool.tile([LH, G], f32, tag="e1")
                        if abl != "noact":
                            nc.scalar.activation(r1[:], za[:], Act.Relu,
                                                 bias=wsb[f"rb1{t}"][:],
                                                 scale=1.0)
                            nc.scalar.activation(t1[:], za[:], Act.Exp,
                                                 bias=wsb[f"eb1{t}"][:],
                                                 scale=INV_ALPHA)
                        if abl != "nodve":
                            nc.vector.tensor_scalar(v1[:], t1[:], ALPHA, ALPHA,
                                                    Alu.min, Alu.subtract)
                        e1 = None
                    else:
                        e1 = e1_pool.tile([LH, G], f32, tag="e1")
                        celu(za[:], t1[:], v1[:], e1[:], t, 1, LH)

                    e2 = e2_pool.tile([LO, G], f32, tag="e2")
                    mmaj = use_fanin and not STRIDED_RHS
                    if mmaj:
                        # m-major layout: addr = m*npg + n
                        e2r = e2[:].rearrange("p (m n) -> p n m", m=M)
                    else:
                        e2r = e2[:].rearrange("p (n m) -> p n m", m=M)
                    if MERGE2:
                        zb = zb_pool.tile([LO, G], f32, tag="za")
                        for h in range(2):
                            cs = slice(h * 512, (h + 1) * 512)
                            if SPLIT1:
                                nc.tensor.matmul(zb[:, cs], wsb[f"w2{t}"][:],
                                                 r1[:, cs], start=True,
                                                 stop=False)
                                nc.tensor.matmul(zb[:, cs], wsb[f"w2{t}"][:],
                                                 v1[:, cs], start=False,
                                                 stop=True)
                            else:
                                nc.tensor.matmul(zb[:, cs], wsb[f"w2{t}"][:],
                                                 e1[:, cs], start=True,
                                                 stop=True)
                        t2 = t2_pool.tile([LO, G], f32, tag="t2")
                        v2 = v2_pool.tile([LO, G], f32, tag="v2")
                        ov = e2r if mmaj else e2[:]
                        if abl != "noact":
                            nc.scalar.activation(t2[:], zb[:], Act.Exp,
                                                 bias=wsb[f"eb2{t}"][:],
                                                 scale=INV_ALPHA)
                        if abl != "nodve":
                            nc.vector.tensor_scalar(v2[:], t2[:], ALPHA, ALPHA,
                                                    Alu.min, Alu.subtract)
                            nc.vector.scalar_tensor_tensor(ov, zb[:],
                                                           wsb[f"rb2{t}"][:],
                                                           v2[:], Alu.add,
                                                           Alu.max)
                    else:
                        for h in range(2):
                            cs = slice(h * 512, (h + 1) * 512)
                            zb = zb_pool.tile([LO, 512], f32, tag="zb")
                            if SPLIT1:
                                nc.tensor.matmul(zb[:], wsb[f"w2{t}"][:],
                                                 r1[:, cs], start=True,
                                                 stop=False)
                                nc.tensor.matmul(zb[:], wsb[f"w2{t}"][:],
                                                 v1[:, cs], start=False,
                                                 stop=True)
                            else:
                                nc.tensor.matmul(zb[:], wsb[f"w2{t}"][:],
                                                 e1[:, cs], start=True,
                                                 stop=True)
                            t2 = t2_pool.tile([LO, 512], f32, tag="t2")
                            v2 = v2_pool.tile([LO, 512], f32, tag="v2")
                            if mmaj:
                                ov = e2r[:, h * (npg // 2):(h + 1) * (npg // 2), :]
                            else:
                                ov = e2[:, cs]
                            eb = wsb[f"eb2{t}"]
                            rb = wsb[f"rb2{t}"]
                            if abl != "noact":
                                nc.scalar.activation(t2[:], zb[:], Act.Exp,
                                                     bias=eb[:], scale=INV_ALPHA)
                            if abl != "nodve":
                                ts_eng = nc.gpsimd if ts_gpsimd else nc.vector
                                ts_eng.tensor_scalar(v2[:], t2[:], ALPHA, ALPHA,
                                                     Alu.min, Alu.subtract)
                                nc.vector.scalar_tensor_tensor(ov, zb[:], rb[:],
                                                               v2[:], Alu.add,
                                                               Alu.max)

                    if use_fanin:
                        # accumulate sum_m (wc1.T @ e2[:, (n,m)]) into
                        # trunk[:, g*npg : (g+1)*npg] via repeated out AP
                        tv = trunk[:, g * npg:(g + 1) * npg]
                        fan_ap = bass.AP(tensor=tv.tensor, offset=tv.offset,
                                         ap=[list(tv.ap[0]), [0, M // 2],
                                             list(tv.ap[-1])])
                        if STRIDED_RHS:
                            # iterate m outer / n inner over the plain layout
                            e2mn = e2[:].rearrange("p (n m) -> p m n", m=M)
                        for mh in range(2):
                            if STRIDED_RHS:
                                rhs = e2mn[:, mh * (M // 2):(mh + 1) * (M // 2), :]
                            else:
                                rhs = e2[:, mh * 512:(mh + 1) * 512]
                            nc.tensor.matmul(fan_ap, wsb["wc1"][:], rhs,
                                             start=False, stop=False,
                                             skip_group_check=True)
                    else:
                        red = small.tile([LO, npg], f32, tag="red")
                        nc.vector.tensor_reduce(red[:], e2r,
                                                axis=mybir.AxisListType.X,
                                                op=Alu.add)
                        asl = atm_sb[:, p * PH + g * npg:p * PH + (g + 1) * npg]
                        nc.vector.tensor_add(asl, asl, red[:])

            # ---- trunk chain for this phase ----
            if use_fanin:
                z1c = trunk[:]
            else:
                z1c_ps = tr_pool.tile([C1, PH], f32, tag="trunk")
                for h in range(PH // 512 if PH > 512 else 1):
                    pass
                nc.tensor.matmul(z1c_ps[:], wsb["wc1"][:], atm_sb[:, nsl],
                                 start=True, stop=True)
                z1c = z1c_ps[:]

            tc1 = t2_pool.tile([C1, PH], f32, tag="t2")
            vc1 = v2_pool.tile([C1, PH], f32, tag="v2")
            ec1 = e1_pool.tile([C1, PH], f32, tag="e1")
            celu(z1c, tc1[:], vc1[:], ec1[:], None, 1, C1)

            z2c = zb_pool.tile([C2, PH], f32, tag=ztag)
            nc.tensor.matmul(z2c[:], wsb["wc2"][:], ec1[:], start=True, stop=True)
            tc2 = t2_pool.tile([C2, PH], f32, tag="t2")
            vc2 = v2_pool.tile([C2, PH], f32, tag="v2")
            ec2 = e1_pool.tile([C2, PH], f32, tag="e1")
            celu(z2c[:], tc2[:], vc2[:], ec2[:], None, 2, C2)

            z3c = zb_pool.tile([C3, PH], f32, tag=ztag)
            nc.tensor.matmul(z3c[:], wsb["wc3"][:], ec2[:], start=True, stop=True)
            tc3 = t2_pool.tile([C3, PH], f32, tag="t2")
            vc3 = v2_pool.tile([C3, PH], f32, tag="v2")
            ec3 = e1_pool.tile([C3, PH], f32, tag="e1")
            celu(z3c[:], tc3[:], vc3[:], ec3[:], None, 3, C3)

            z4c = zb_pool.tile([1, PH], f32, tag=ztag)
            nc.tensor.matmul(z4c[:], wsb["wc4"][:], ec3[:], start=True, stop=True)
            o = small.tile([1, PH], f32, tag="o")
            nc.scalar.activation(o[:], z4c[:], Act.Identity,
                                 bias=wsb["bc4"][:], scale=1.0)
            om = small.tile([1, PH], f32, tag="om")
            nc.vector.tensor_mul(om[:], o[:], mi_sb[:, nsl])
            nc.sync.dma_start(out_dram[:, nsl], om[:])

    nc.compile()
    return nc


# --------------------------------------------------------------------------
# host side
# --------------------------------------------------------------------------

def _celu_np(x):
    x = x.astype(np.float32)
    return (np.maximum(x, 0.0)
            + np.minimum(0.0, np.float32(ALPHA)
                         * np.expm1(x * np.float32(INV_ALPHA)))).astype(np.float32)


def _prep_core(inputs, c, nmol=BL):
    s = slice(c * nmol, (c + 1) * nmol)
    nodes = nmol * N
    d = {}
    for t, key in (("j", "aj"), ("k", "ak"), ("l", "al")):
        flat = np.ascontiguousarray(inputs[key][s], dtype=np.float32).reshape(-1, 3)
        d[f"x{t}"] = np.ascontiguousarray(flat.T)
    ai = np.ascontiguousarray(inputs["ai"][s], dtype=np.float32).reshape(-1, 3)
    d["xi"] = np.ascontiguousarray(ai.T)
    mi = ((ai[:, 0] + ai[:, 1]) + ai[:, 2]) != 0
    d["mi"] = mi.astype(np.float32)[None, :]

    corr = np.zeros((nodes, LO), np.float32)
    for key, wn in (("aj", "j"), ("ak", "k"), ("al", "l")):
        flat = np.ascontiguousarray(inputs[key][s], dtype=np.float32).reshape(-1, 3)
        ssum = (flat[:, 0] + flat[:, 1]) + flat[:, 2]
        idx = np.nonzero(ssum == 0)[0]
        if idx.size:
            W1 = inputs[f"W{wn}1"].astype(np.float32)
            b1 = inputs[f"b{wn}1"].astype(np.float32)
            W2 = inputs[f"W{wn}2"].astype(np.float32)
            b2 = inputs[f"b{wn}2"].astype(np.float32)
            h1 = _celu_np(flat[idx] @ W1.T + b1)
            h2 = _celu_np(h1 @ W2.T + b2)
            np.subtract.at(corr, idx // M, h2)
    d["corr"] = np.ascontiguousarray(corr.T)

    def ct(a):
        return np.ascontiguousarray(a, dtype=np.float32)

    for t, wn in (("j", "j"), ("k", "k"), ("l", "l"), ("i", "i")):
        W1 = inputs[f"W{wn}1"]
        b1 = inputs[f"b{wn}1"].astype(np.float32)
        W2 = inputs[f"W{wn}2"]
        b2 = inputs[f"b{wn}2"].astype(np.float32)
        d[f"w1{t}"] = ct(W1.T)
        d[f"w2{t}"] = ct(W2.T)
        d[f"eb1{t}"] = ct(b1 * INV_ALPHA + LN_ALPHA)[:, None]
        d[f"rb1{t}"] = ct(b1)[:, None]
        d[f"eb2{t}"] = ct(b2 * INV_ALPHA + LN_ALPHA)[:, None]
        d[f"rb2{t}"] = ct(b2)[:, None]
    for i, (wk, bk) in enumerate((("W1", "b1"), ("W2", "b2"),
                                  ("W3", "b3"), ("W4", "b4")), start=1):
        W = inputs[wk]
        bb = inputs[bk].astype(np.float32)
        d[f"wc{i}"] = ct(W.T)
        if i < 4:
            d[f"ebc{i}"] = ct(bb * INV_ALPHA + LN_ALPHA)[:, None]
            d[f"rbc{i}"] = ct(bb)[:, None]
        else:
            d["bc4"] = ct(bb)[:, None]
    return d


def _get_program(nmol=BL):
    key = (nmol, USE_FANIN)
    if key not in _PROGRAM_CACHE:
        _PROGRAM_CACHE[key] = _build_program(nmol=nmol, use_fanin=USE_FANIN)
    return _PROGRAM_CACHE[key]


def run(inputs, trace=False, **kwargs):
    """Returns (full_output [B,N,1] f32, BassKernelResults)."""
    from concourse.bass_utils import run_bass_kernel_spmd
    inputs = {k: np.asarray(v) for k, v in inputs.items()}
    nc = _get_program()
    in_maps = [_prep_core(inputs, c) for c in range(NCORES)]
    res = run_bass_kernel_spmd(nc, in_maps, core_ids=list(range(NCORES)),
                               trace=trace, **kwargs)
    outs = [res.results[c]["out"].reshape(BL, N, 1) for c in range(NCORES)]
    return np.concatenate(outs, axis=0).astype(np.float32), res


def kernel(**inputs):
    out, _ = run(inputs)
    return out
